# revision 17
# baseline (speedup 1.0000x reference)
"""Context-aware attention kernel for 8 Trainium2 NeuronCores.

Reference computation (B=128, LQ=32, LK=128, D=1024, H=16, DK=64):
  K_cat = concat(K_seq, Q_seq) on the sequence axis  -> [B, 160, D]
  Qh/Kh/Vh = per-head projections, custom exp-based masked attention
  out[b,q,:] = (sum_k mexp[q,k] Vh[k,:]) / (sum_k mexp[q,k] + 1e-8)
  with mexp = exp(QK^T/8) * mask.

Sharding: data-parallel over batch, 16 batches per core, processed as 4
quads of 4 batches (each quad packs 4*32 query rows into 128 partitions).

Schedule: ONE dense PE stream. The projection GEMMs of all 4 quads are
emitted back to back; the small attention matmuls are drained from a
FIFO work queue between projection matmuls (1-2 per slot). Attention
items are released as soon as the projection units they depend on have
been emitted (plus a small slot slack so the scalar/vector exp+mask
chain is done before the PE reaches the consuming matmul). This keeps
the tensor engine continuously busy with mostly-dense work, which holds
the PE clock at its top p-state (idle gaps or sparse-only stretches
drop it to half speed for multiple microseconds). The last quad's
projection units are ordered so most of its attention interleaves too,
leaving only a tiny filler-padded tail.
"""

import os
import sys

for _p in ("/opt/trn_rl_repo", "/root/.axon_site/_ro/trn_rl_repo"):
    if os.path.isdir(_p) and _p not in sys.path:
        sys.path.insert(0, _p)

import numpy as np
import ml_dtypes

import concourse.bacc as bacc
import concourse.mybir as mybir
import concourse.tile as tile
from concourse.bass_utils import run_bass_kernel_spmd

B, LQ, LK, D, H, DK = 128, 32, 128, 1024, 16, 64
L = LK + LQ              # 160 keys per batch after concat
NCORES = 8
NB = B // NCORES         # 16 batches per core
QUAD = 4                 # batches packed into one 128-partition group
NQ = NB // QUAD          # 4 quads per core
HC = DK + 1              # per-head V columns incl. the ones column
SCALE = 1.0 / np.sqrt(float(DK))

BF = mybir.dt.bfloat16
F32 = mybir.dt.float32
EXP = mybir.ActivationFunctionType.Exp

_NC = None
_LAST_RESULT = None


def _build():
    nc = bacc.Bacc(
        "TRN2",
        target_bir_lowering=False,
        debug=False,
        enable_asserts=False,
        num_devices=NCORES,
    )
    xt = nc.dram_tensor("xt", [D, NB * L], BF, kind="ExternalInput").ap()
    xvt = nc.dram_tensor("xvt", [D, NB * LK], BF, kind="ExternalInput").ap()
    xqt = nc.dram_tensor("xqt", [D, NB * LQ], BF, kind="ExternalInput").ap()
    wqt = nc.dram_tensor("wqt", [D, D], BF, kind="ExternalInput").ap()
    wkt = nc.dram_tensor("wkt", [D, D], BF, kind="ExternalInput").ap()
    wvt = nc.dram_tensor("wvt", [D, D], BF, kind="ExternalInput").ap()
    bqd = nc.dram_tensor("bq", [8, 128, 1], F32, kind="ExternalInput").ap()
    bkd = nc.dram_tensor("bk", [8, 128, 1], F32, kind="ExternalInput").ap()
    bvd = nc.dram_tensor("bvrow", [1, D], BF, kind="ExternalInput").ap()
    mad = nc.dram_tensor("maska", [NQ, LK, QUAD * LQ], BF, kind="ExternalInput").ap()
    mbd = nc.dram_tensor("maskb", [NQ, QUAD * LQ, LQ], BF, kind="ExternalInput").ap()
    outd = nc.dram_tensor("out", [NB * LQ, D], F32, kind="ExternalOutput").ap()

    from contextlib import ExitStack

    with tile.TileContext(nc) as tc, ExitStack() as st:
        pers = st.enter_context(tc.tile_pool(name="pers", bufs=1))
        xtp = st.enter_context(tc.tile_pool(name="xtp", bufs=16))
        khtp = st.enter_context(tc.tile_pool(name="khtp", bufs=3))
        qhtp = st.enter_context(tc.tile_pool(name="qhtp", bufs=3))
        vhap = st.enter_context(tc.tile_pool(name="vhap", bufs=12))
        vhbp = st.enter_context(tc.tile_pool(name="vhbp", bufs=3))
        mapl = st.enter_context(tc.tile_pool(name="mapl", bufs=3))
        mbpl = st.enter_context(tc.tile_pool(name="mbpl", bufs=3))
        meap = st.enter_context(tc.tile_pool(name="meap", bufs=6))
        mebp = st.enter_context(tc.tile_pool(name="mebp", bufs=6))
        outp = st.enter_context(tc.tile_pool(name="outp", bufs=3))
        mscp = st.enter_context(tc.tile_pool(name="mscp", bufs=8))
        pproj = st.enter_context(tc.tile_pool(name="pproj", bufs=2, space="PSUM"))
        psa = st.enter_context(tc.tile_pool(name="psa", bufs=2, space="PSUM"))
        psb = st.enter_context(tc.tile_pool(name="psb", bufs=1, space="PSUM"))
        pctx = st.enter_context(tc.tile_pool(name="pctx", bufs=2, space="PSUM"))
        pfill = st.enter_context(tc.tile_pool(name="pfill", bufs=1, space="PSUM"))

        # ---- tiny constant DMAs first (they unblock the PE stream) -----
        # tiny DMAs go on the (otherwise idle) gpsimd queue so they don't
        # serialize ahead of the big weight/input loads on sync/scalar
        bq_sb, bk_sb = [], []
        for o in range(8):
            t = pers.tile([128, 1], F32, name=f"bqs{o}", tag=f"bqs{o}")
            nc.gpsimd.dma_start(t[:], bqd[o])
            bq_sb.append(t)
            t = pers.tile([128, 1], F32, name=f"bks{o}", tag=f"bks{o}")
            nc.gpsimd.dma_start(t[:], bkd[o])
            bk_sb.append(t)
        ones1 = pers.tile([1, 128], BF, tag="ones1")
        nc.vector.memset(ones1[:], 1.0)
        bvr = pers.tile([1, D], BF, tag="bvr")
        nc.gpsimd.dma_start(bvr[:], bvd[:])
        bvb = pers.tile([128, D], F32, tag="bvb")
        bvb_v = bvb.rearrange("p (h c) -> p h c", c=DK)

        # ---- input tiles ------------------------------------------------
        xin_cache = {}

        def _quad_inputs(gq, only=None):
            if gq in xin_cache:
                return xin_cache[gq]
            c0 = gq * QUAD * L
            cv0 = gq * QUAD * LK
            cq0 = gq * QUAD * LQ
            xt_sb = [xtp.tile([128, QUAD * L], BF, name="xt", tag="xt") for _ in range(8)]
            xq_sb = [xtp.tile([128, QUAD * LQ], BF, name="xq", tag="xq", bufs=24) for _ in range(8)]
            xv_sb = [xtp.tile([128, QUAD * LK], BF, name="xv", tag="xv") for _ in range(8)]
            xte = nc.scalar if gq == 0 else nc.sync
            for d in range(8):
                xte.dma_start(xt_sb[d][:], xt[d * 128 : (d + 1) * 128, c0 : c0 + QUAD * L])
            for d in range(8):
                xte.dma_start(xq_sb[d][:], xqt[d * 128 : (d + 1) * 128, cq0 : cq0 + QUAD * LQ])
            if only != "tq":
                for d in range(8):
                    nc.sync.dma_start(xv_sb[d][:], xvt[d * 128 : (d + 1) * 128, cv0 : cv0 + QUAD * LK])
            xin_cache[gq] = (xt_sb, xq_sb, xv_sb)
            return xin_cache[gq]

        def _quad_inputs_xv(gq):
            # late xv DMA for quad 0 (so wq streams in before xv/wv)
            xt_sb, xq_sb, xv_sb = xin_cache[gq]
            cv0 = gq * QUAD * LK
            for d in range(8):
                nc.sync.dma_start(xv_sb[d][:], xvt[d * 128 : (d + 1) * 128, cv0 : cv0 + QUAD * LK])

        # ---- weights: wk, quad-0 xt/xq, wq, quad-0 xv, wv ---------------
        wq_sb, wk_sb, wv_sb = [], [], []
        for lst, nm in ((wk_sb, "wk"), (wq_sb, "wq"), (wv_sb, "wv")):
            for d in range(8):
                lst.append(pers.tile([128, D], BF, name=f"{nm}{d}", tag=f"{nm}{d}"))
        for d in range(8):
            nc.sync.dma_start(wk_sb[d][:], wkt[d * 128 : (d + 1) * 128, :])
        _quad_inputs(0, only="tq")
        for d in range(8):
            nc.gpsimd.dma_start(wq_sb[d][:], wqt[d * 128 : (d + 1) * 128, :])
        _quad_inputs_xv(0)
        for d in range(8):
            nc.gpsimd.dma_start(wv_sb[d][:], wvt[d * 128 : (d + 1) * 128, :])

        warm_refs = {}

        # ---- PE slot accounting + attention work queue ------------------
        SLOT = [0]
        work = []          # list of (min_slot, emit_fn); strict FIFO
        wi = [0]

        def bump():
            SLOT[0] += 1

        def push_items(items, slack):
            ms = SLOT[0] + slack
            for it in items:
                work.append((ms, it))

        def backlog():
            return len(work) - wi[0]

        def drain(r):
            done = 0
            while done < r and wi[0] < len(work):
                ms, fn = work[wi[0]]
                if ms > SLOT[0]:
                    break
                wi[0] += 1
                fn()
                done += 1

        fill_box = {}

        def _filler():
            # dense dummy matmul group to keep the PE busy when no real
            # dense work is available. Writes a dedicated never-read PSUM
            # bank: no vector copy-out, so fillers can never stall behind
            # a congested vector queue (only a PE-local WAW dep).
            if "ps" not in fill_box:
                fill_box["ps"] = pfill.tile([128, 512], F32, name="fill", tag="fill")
            ps = fill_box["ps"]
            for i in range(3):
                nc.tensor.matmul(ps[:], warm_refs["a"][:], warm_refs["b"][:],
                                 start=(i == 0), stop=(i == 2),
                                 skip_group_check=True)
                bump()

        def _emit_warmup_bvb(groups=10):
            # Full-array warm-up matmuls on memset data: ramps the PE
            # p-state and covers the initial weight/input DMA latency.
            wma = pers.tile([128, 128], BF, tag="wma")
            nc.vector.memset(wma[:], 1.0 / 128.0)
            wmb = pers.tile([128, 512], BF, tag="wmb")
            nc.vector.memset(wmb[:], 1.0)
            warm_refs.update({"a": wma, "b": wmb})
            for g in range(groups):
                _filler()
                _filler()
            # broadcast bv across partitions via K=1 matmuls with ones
            for oc in range(2):
                ps = pproj.tile([128, 512], F32, name="warm2", tag="proj")
                nc.tensor.matmul(ps[:], ones1[:], bvr[:, oc * 512 : (oc + 1) * 512],
                                 start=True, stop=True)
                nc.vector.tensor_copy(bvb[:, oc * 512 : (oc + 1) * 512], ps[:])

        # ---- per-quad state ---------------------------------------------
        ST = {}

        def _quad_state(gq):
            if gq in ST:
                return ST[gq]
            s = {
                "kht": [khtp.tile([128, QUAD * L], BF, name=f"kht{o}", tag=f"kht{o}") for o in range(8)],
                "qht": [qhtp.tile([128, QUAD * LQ], BF, name=f"qht{o}", tag=f"qht{o}") for o in range(8)],
                "vha": {}, "vhb": None, "ma": None, "mb": None,
                # attention bookkeeping
                "outq": None, "pend": {}, "done": [0],
                "vready": {0: False, 1: False},
                "pending_ctx": {0: [], 1: []},
            }
            t = mapl.tile([LK, QUAD * LQ], BF, name="ma", tag="ma")
            nc.sync.dma_start(t[:], mad[gq])
            s["ma"] = t
            t = mbpl.tile([QUAD * LQ, LQ], BF, name="mb", tag="mb")
            nc.sync.dma_start(t[:], mbd[gq])
            s["mb"] = t
            ST[gq] = s
            return s

        # ---- projection unit item lists (one matmul per item) -----------
        def k_unit_items(gq, o, sub):
            s = _quad_state(gq)
            xt_sb = xin_cache[gq][0]
            box = {}

            def mk(d):
                def run():
                    if d == 0:
                        box["ps"] = pproj.tile([128, 512], F32, name="kps", tag="proj")
                    ps = box["ps"]
                    nc.tensor.matmul(
                        ps[:, 0:320],
                        wk_sb[d][:, o * 128 : (o + 1) * 128],
                        xt_sb[d][:, sub * 320 : (sub + 1) * 320],
                        start=(d == 0), stop=(d == 7),
                    )
                    bump()
                    if d == 7:
                        nc.vector.tensor_scalar_add(
                            s["kht"][o][:, sub * 320 : (sub + 1) * 320],
                            ps[:, 0:320], bk_sb[o][:],
                        )
                return run

            return [mk(d) for d in range(8)]

        def q_unit_items(gq, o):
            s = _quad_state(gq)
            xq_sb = xin_cache[gq][1]
            box = {}

            def mk(d):
                def run():
                    if d == 0:
                        box["ps"] = pproj.tile([128, 512], F32, name="qps", tag="proj")
                    ps = box["ps"]
                    nc.tensor.matmul(
                        ps[:, 0 : QUAD * LQ],
                        wq_sb[d][:, o * 128 : (o + 1) * 128],
                        xq_sb[d][:], start=(d == 0), stop=(d == 7),
                    )
                    bump()
                    if d == 7:
                        nc.vector.tensor_scalar_add(
                            s["qht"][o][:], ps[:, 0 : QUAD * LQ], bq_sb[o][:]
                        )
                return run

            return [mk(d) for d in range(8)]

        def va_unit_items(gq, j, oc):
            s = _quad_state(gq)
            xv_sb = xin_cache[gq][2]
            box = {}
            gb = gq * QUAD + j

            def mk(d):
                def run():
                    if d == 0:
                        if oc == 0:
                            va = vhap.tile([128, H * HC], BF, name="vha", tag="vha")
                            vav = va.rearrange("p (h c) -> p h c", c=HC)
                            nc.vector.memset(vav[:, :, DK : DK + 1], 1.0)
                            s["vha"][gb] = va
                        box["ps"] = pproj.tile([128, 512], F32, name="vps", tag="proj")
                    ps = box["ps"]
                    nc.tensor.matmul(
                        ps[:],
                        xv_sb[d][:, j * LK : (j + 1) * LK],
                        wv_sb[d][:, oc * 512 : (oc + 1) * 512],
                        start=(d == 0), stop=(d == 7),
                    )
                    bump()
                    if d == 7:
                        va = s["vha"][gb]
                        vav = va.rearrange("p (h c) -> p h c", c=HC)
                        nc.vector.tensor_add(
                            vav[:, oc * 8 : (oc + 1) * 8, 0:DK],
                            ps[:].rearrange("p (h c) -> p h c", c=DK),
                            bvb_v[:, oc * 8 : (oc + 1) * 8, :],
                        )
                return run

            return [mk(d) for d in range(8)]

        def vb_unit_items(gq, oc):
            s = _quad_state(gq)
            xq_sb = xin_cache[gq][1]
            box = {}

            def mk(d):
                def run():
                    if d == 0:
                        if oc == 0:
                            vb = vhbp.tile([128, H * HC], BF, name="vhb", tag="vhb")
                            vbv = vb.rearrange("p (h c) -> p h c", c=HC)
                            nc.vector.memset(vbv[:, :, DK : DK + 1], 1.0)
                            s["vhb"] = vb
                        box["ps"] = pproj.tile([128, 512], F32, name="vbps", tag="proj")
                    ps = box["ps"]
                    nc.tensor.matmul(
                        ps[:], xq_sb[d][:],
                        wv_sb[d][:, oc * 512 : (oc + 1) * 512],
                        start=(d == 0), stop=(d == 7),
                    )
                    bump()
                    if d == 7:
                        vb = s["vhb"]
                        vbv = vb.rearrange("p (h c) -> p h c", c=HC)
                        nc.vector.tensor_add(
                            vbv[:, oc * 8 : (oc + 1) * 8, 0:DK],
                            ps[:].rearrange("p (h c) -> p h c", c=DK),
                            bvb_v[:, oc * 8 : (oc + 1) * 8, :],
                        )
                return run

            return [mk(d) for d in range(8)]

        # ---- attention item lists ---------------------------------------
        def ctx_items(gq, h):
            s = _quad_state(gq)
            box = {}

            def mk(i):
                j, kind = i // 2, i % 2
                gb = gq * QUAD + j

                def run():
                    if i == 0:
                        box["mea"], box["meb"] = s["pend"].pop(h)
                        box["ps"] = pctx.tile([128, HC], F32, name="ctx", tag="ctx")
                    ctxp = box["ps"]
                    if kind == 0:
                        nc.tensor.matmul(
                            ctxp[32 * j : 32 * (j + 1), :],
                            box["mea"][:, 32 * j : 32 * (j + 1)],
                            s["vha"][gb][:, h * HC : (h + 1) * HC],
                            start=True, stop=False, tile_position=(0, 32 * j),
                        )
                    else:
                        nc.tensor.matmul(
                            ctxp[32 * j : 32 * (j + 1), :],
                            box["meb"][32 * j : 32 * (j + 1), :],
                            s["vhb"][32 * j : 32 * (j + 1), h * HC : (h + 1) * HC],
                            start=False, stop=True, tile_position=(32 * j, 32 * j),
                        )
                    bump()
                    if i == 7:
                        r = mscp.tile([128, 1], F32, name="r", tag="r")
                        nc.vector.tensor_scalar_add(r[:], ctxp[:, DK : DK + 1], 1e-8)
                        nc.vector.reciprocal(r[:], r[:])
                        nc.vector.tensor_scalar_mul(
                            s["outq"][:, h * DK : (h + 1) * DK], ctxp[:, 0:DK], r[:]
                        )
                        s["done"][0] += 1
                        # ship each half of the output as soon as its 8
                        # heads are normalized (shortens the final drain)
                        if s["done"][0] == H // 2:
                            nc.sync.dma_start(
                                outd[gq * 128 : (gq + 1) * 128, 0 : D // 2],
                                s["outq"][:, 0 : D // 2],
                            )
                        elif s["done"][0] == H:
                            nc.sync.dma_start(
                                outd[gq * 128 : (gq + 1) * 128, D // 2 : D],
                                s["outq"][:, D // 2 : D],
                            )
                return run

            return [mk(i) for i in range(8)]

        def score_items(gq, h):
            s = _quad_state(gq)
            ot, h2 = h // 2, 64 * (h % 2)
            box = {}

            def mk(i):
                j, kind = i // 2, i % 2

                def run():
                    if i == 0:
                        if s["outq"] is None:
                            s["outq"] = outp.tile([128, D], F32, name="outq", tag="outq")
                        box["sa"] = psa.tile([LK, QUAD * LQ], F32, name="sa", tag="sa")
                        box["sb"] = psb.tile([QUAD * LQ, LQ], F32, name="sb", tag="sb")
                    if kind == 0:
                        nc.tensor.matmul(
                            box["sa"][:, 32 * j : 32 * (j + 1)],
                            s["kht"][ot][h2 : h2 + 64, j * L : j * L + LK],
                            s["qht"][ot][h2 : h2 + 64, j * LQ : (j + 1) * LQ],
                            start=True, stop=True, tile_position=(h2, 0),
                        )
                    else:
                        nc.tensor.matmul(
                            box["sb"][32 * j : 32 * (j + 1), :],
                            s["kht"][ot][h2 : h2 + 64, j * L + LK : (j + 1) * L],
                            s["qht"][ot][h2 : h2 + 64, j * LQ : (j + 1) * LQ],
                            start=True, stop=True, tile_position=(h2, 32 * j),
                        )
                    bump()
                    if i == 7:
                        mea = meap.tile([LK, QUAD * LQ], BF, name="mea", tag="mea")
                        nc.scalar.activation(mea[:], box["sa"][:], EXP, scale=SCALE)
                        nc.vector.tensor_mul(mea[:], mea[:], s["ma"][:])
                        meb = mebp.tile([QUAD * LQ, LQ], BF, name="meb", tag="meb")
                        nc.scalar.activation(meb[:], box["sb"][:], EXP, scale=SCALE)
                        nc.vector.tensor_mul(meb[:], meb[:], s["mb"][:])
                        s["pend"][h] = (mea, meb)
                        oc = h // 8
                        if s["vready"][oc]:
                            push_items(ctx_items(gq, h), slack=12)
                        else:
                            s["pending_ctx"][oc].append(h)
                return run

            return [mk(i) for i in range(8)]

        # boundary actions (no emission, just releases)
        def rel_scores(gq, ot):
            def run():
                push_items(score_items(gq, 2 * ot), slack=8)
                push_items(score_items(gq, 2 * ot + 1), slack=8)
            return run

        def rel_v(gq, oc):
            def run():
                s = _quad_state(gq)
                s["vready"][oc] = True
                for h in s["pending_ctx"][oc]:
                    push_items(ctx_items(gq, h), slack=8)
                s["pending_ctx"][oc] = []
            return run

        # ---- per-quad projection unit order -----------------------------
        def quad_units(gq):
            units = []  # list of (mm_items, boundary_fn_or_None)
            if gq == 0:
                # wk lands first, then wq, then xv0/wv: k-units first.
                for o in range(8):
                    for sub in range(2):
                        units.append((k_unit_items(gq, o, sub), None))
                for o in range(8):
                    units.append((q_unit_items(gq, o), rel_scores(gq, o)))
                for j in range(QUAD):
                    units.append((va_unit_items(gq, j, 0), None))
                units.append((vb_unit_items(gq, 0), rel_v(gq, 0)))
                for j in range(QUAD):
                    units.append((va_unit_items(gq, j, 1), None))
                units.append((vb_unit_items(gq, 1), rel_v(gq, 1)))
            else:
                # all weights resident: release attention deps early
                for j in range(QUAD):
                    units.append((va_unit_items(gq, j, 0), None))
                units.append((vb_unit_items(gq, 0), rel_v(gq, 0)))
                for o in range(4):
                    units.append((k_unit_items(gq, o, 0), None))
                    units.append((k_unit_items(gq, o, 1), None))
                    units.append((q_unit_items(gq, o), rel_scores(gq, o)))
                for j in range(QUAD):
                    units.append((va_unit_items(gq, j, 1), None))
                units.append((vb_unit_items(gq, 1), rel_v(gq, 1)))
                for o in range(4, 8):
                    units.append((k_unit_items(gq, o, 0), None))
                    units.append((k_unit_items(gq, o, 1), None))
                    units.append((q_unit_items(gq, o), rel_scores(gq, o)))
            return units

        # ---- emission ----------------------------------------------------
        _emit_warmup_bvb(groups=10)
        _quad_state(0)
        for gq in range(NQ):
            if gq + 1 < NQ:
                _quad_inputs(gq + 1)
                _quad_state(gq + 1)
            for mms, boundary in quad_units(gq):
                for fn in mms:
                    fn()
                # Drain attention in unit-sized bursts BETWEEN projection
                # units (not between individual matmuls): fine-grained
                # mixing dilutes every HAM activity window below the
                # un-throttle threshold and halves the PE clock for the
                # whole run (measured: 537us vs 373us). Coarse bursts keep
                # most windows dense/warm.
                if gq >= NQ - 2:
                    # last two quads: clear the backlog while dense work
                    # still flows (anything left after proj ends runs cold)
                    drain(24 if backlog() > 64 else 16)
                else:
                    drain(16 if backlog() > 96 else 8)
                if boundary is not None:
                    boundary()

        # tail: drain the remaining attention with dense filler padding
        cnt = 0
        while wi[0] < len(work):
            ms, fn = work[wi[0]]
            if ms > SLOT[0]:
                _filler()
                continue
            wi[0] += 1
            fn()
            cnt += 1
            if cnt % 6 == 0:
                _filler()

    nc.compile()
    return nc


def _get_nc():
    global _NC
    if _NC is None:
        _NC = _build()
    return _NC


def kernel(**inputs):
    global _LAST_RESULT
    Q_seq = np.asarray(inputs["Q_seq"], dtype=np.float32)
    K_seq = np.asarray(inputs["K_seq"], dtype=np.float32)
    V_seq = np.asarray(inputs["V_seq"], dtype=np.float32)
    tm = np.asarray(inputs["title_mask"], dtype=np.float32)
    bm = np.asarray(inputs["body_mask"], dtype=np.float32)
    Wq = np.asarray(inputs["Wq"], dtype=np.float32)
    Wk = np.asarray(inputs["Wk"], dtype=np.float32)
    Wv = np.asarray(inputs["Wv"], dtype=np.float32)
    bq = np.asarray(inputs["bq"], dtype=np.float32)
    bk = np.asarray(inputs["bk"], dtype=np.float32)
    bv = np.asarray(inputs["bv"], dtype=np.float32)

    bf = ml_dtypes.bfloat16
    # K_cat = concat(K_seq, Q_seq); V_cat = concat(V_seq, Q_seq). The V
    # projection of the shared Q_seq rows reuses xt's query columns, so
    # xvt only carries the V_seq part.
    Xk = np.concatenate([K_seq, Q_seq], axis=1)  # [B, L, D]

    wqt = np.ascontiguousarray(Wq.T).astype(bf)
    wkt = np.ascontiguousarray(Wk.T).astype(bf)
    wvt = np.ascontiguousarray(Wv.T).astype(bf)

    maska_bt = (bm * tm[:, :, None]).transpose(0, 2, 1)  # [B,128,32]
    maska = np.ascontiguousarray(
        maska_bt.reshape(B // QUAD, QUAD, LK, LQ).transpose(0, 2, 1, 3).reshape(B // QUAD, LK, QUAD * LQ)
    ).astype(bf)  # [B/4, 128, 4*32]
    maskb = (tm[:, :, None] * tm[:, None, :]).astype(bf)  # [B, 32(i), 32(q)]

    nc = _get_nc()
    in_maps = []
    for c in range(NCORES):
        sl = slice(c * NB, (c + 1) * NB)
        XT = np.ascontiguousarray(Xk[sl].reshape(NB * L, D).T).astype(bf)
        XVT = np.ascontiguousarray(V_seq[sl].reshape(NB * LK, D).T).astype(bf)
        XQT = np.ascontiguousarray(Q_seq[sl].reshape(NB * LQ, D).T).astype(bf)
        in_maps.append({
            "xt": XT,
            "xvt": XVT,
            "xqt": XQT,
            "wqt": wqt, "wkt": wkt, "wvt": wvt,
            "bq": np.ascontiguousarray(bq.reshape(8, 128, 1)),
            "bk": np.ascontiguousarray(bk.reshape(8, 128, 1)),
            "bvrow": np.ascontiguousarray(bv.reshape(1, D)).astype(bf),
            "maska": np.ascontiguousarray(maska[c * NB // QUAD : (c + 1) * NB // QUAD]),
            "maskb": np.ascontiguousarray(maskb[sl].reshape(NB // QUAD, QUAD * LQ, LQ)),
        })

    res = run_bass_kernel_spmd(nc, in_maps, core_ids=list(range(NCORES)))
    _LAST_RESULT = res
    out = np.concatenate(
        [res.results[c]["out"].reshape(NB, LQ, D) for c in range(NCORES)], axis=0
    )
    return np.ascontiguousarray(out.astype(np.float32))
